# revision 55
# baseline (speedup 1.0000x reference)
"""RBF-kernel attention (nn_Attention_76081050682051) on 8 TRN2 NeuronCores.

Self-contained Bass/Tile kernel. `kernel(**inputs)` takes the FULL unsharded
inputs of reference.setup_inputs() and returns the FULL [4, 2048, 256] f32
output.

Sharding (B x tensor-parallel heads): core c -> batch b = c//2, heads
[4*(c%2), 4*(c%2)+4); pairwise AllReduce ([0,1],[2,3],[4,5],[6,7]) combines
the two half-head partial outputs of each batch after the W_o projection.

Device math:
  LayerNorm per-partition via bn_stats/bn_aggr; rsqrt via DVE reciprocal +
  2 Newton steps (Pool engine); xnT blocks via PE transposes (f32r).
  Per head: K'T/Q'T = (folded W).T @ xnT with sqrt(2*gamma)*ln_scale folded
  into W_q/W_k on the host.  For heads 1-3 the K'/Q' blocks are split into
  fp8e4 hi+lo pairs and the S x S logits run as 3 DoubleRow fp8 matmuls
  (hi*hi + hi*lo + lo*hi, 256-deep contraction per pass) -- 0.75x the f32r
  cycle count at ~6e-3 relative error.  Head 0 stays f32r so its warmup
  overlaps the LayerNorm chain without extra DVE/ACT queue pressure.
  k2/q2 row sums: DVE/Pool squares the projection PSUM, then a [128,1]
  matmul with the squared tile as the *stationary* operand reduces 128
  t-values per instruction; results land per-partition, exactly the layout
  the ACT bias (k2) and the post-W_o scale (q2) need -- no DRAM round trip.
  scoresT[t, s] = exp(qk'[t,s] - k2'[t]/2) via one ACT op per [128,512]
  tile; exp(-q2'[s]/2) is applied after W_o as a per-partition scale.
  outT = V.T @ scoresT accumulates over t in PSUM; W_o runs on outT column
  slices; partial outputs AllReduce (4 quarter-chunks) within each batch
  pair.  Emission is software-pipelined across heads.
"""
import sys
sys.path.insert(0, '/opt/trn_rl_repo')
import numpy as np
from concourse import bass, bacc, tile, mybir, masks
from concourse.bass_utils import run_bass_kernel_spmd

F32 = mybir.dt.float32
F32R = mybir.dt.float32r
FP8 = mybir.dt.float8e4
AF = mybir.ActivationFunctionType
OP = mybir.AluOpType
DR = mybir.MatmulPerfMode.DoubleRow

B, S, E, H = 4, 2048, 256, 8
HL = 4          # heads per core
EC = 2          # e chunks of 128
SB = 4          # s blocks of 512
ST = 16         # s/t tiles of 128
N_CORES = 8
EPS = 1e-5

NO_COLL = False
N_HEADS_BUILD = HL


def build_kernel(debug=False):
    nc = bacc.Bacc("TRN2", target_bir_lowering=False, debug=False,
                   num_devices=N_CORES)

    x_ext = nc.declare_dram_parameter("x", [S, E], F32, isOutput=False)
    w_ext = {}
    for wname in ("wq", "wk", "wv", "wo"):
        w_ext[wname] = nc.declare_dram_parameter(wname, [HL, 128, EC * E], F32,
                                                 isOutput=False)
    # fp8 hi/lo weight pairs (heads 1..3), pre-scaled by 2^8 on the host so
    # the folded weights clear e4m3's subnormal band; stored as uint8 bytes
    for wname in ("wq", "wk", "wv"):
        for part in ("h", "l"):
            n = f"{wname}{part}8"
            w_ext[n] = nc.declare_dram_parameter(n, [HL, 128, EC, E],
                                                 mybir.dt.uint8, isOutput=False)
    out_ext = nc.declare_dram_parameter("out", [S, E], F32, isOutput=True)

    with tile.TileContext(nc) as tc:
        with tc.tile_pool(name="sb", bufs=1) as sb, \
             tc.tile_pool(name="sbt", bufs=1) as sbt, \
             tc.tile_pool(name="ps", bufs=1, space="PSUM") as ps, \
             tc.tile_pool(name="dram", bufs=1, space="DRAM") as dram:

            # ---------- constants ----------
            ones32 = sb.tile([128, 2], F32, name="ones32")
            nc.any.memset(ones32[:], 1.0)
            ones_col = sb.tile([128, 2], F32R, name="ones_col")
            nc.vector.tensor_copy(ones_col[:], ones32[:])
            ident128 = sb.tile([128, 128], F32, name="ident128")
            masks.make_identity(nc, ident128[:])
            # pre-warm the PE pstate ramp clock with cheap transposes so the
            # first real transposes/matmuls run at full frequency
            warm = ps.tile([128, 128], F32, name="warm", tag="stps", bufs=2)
            for _ in range(12):
                nc.tensor.transpose(warm[:32, :32], ident128[:32, :32],
                                    ident128[:32, :32])

            # ---------- x + head-0 weight loads (queue order matters) ----------
            xu_tiles = []
            wtmp0 = {}

            def load_w(h, names, store):
                for wname in names:
                    wt = sbt.tile([128, EC * E], F32, name=f"w_{wname}",
                                  tag="wtmp", bufs=4)
                    nc.sync.dma_start(wt[:], w_ext[wname][h])
                    store[wname] = wt

            def load_w8(h, store):
                for wname in ("wq", "wk", "wv"):
                    for part in ("h", "l"):
                        n = f"{wname}{part}8"
                        wt = sbt.tile([128, EC, E], mybir.dt.uint8,
                                      name=f"w_{n}", tag="w8", bufs=12)
                        nc.sync.dma_start(wt[:], w_ext[n][h])
                        store[n] = wt

            for sbk in range(SB):
                xu = sbt.tile([128, 4 * E], F32, name="xu", tag="xu", bufs=4)
                xv = xu[:].rearrange("p (t e) -> p t e", t=4)
                src = x_ext[sbk * 512:(sbk + 1) * 512, :] \
                    .rearrange("(t p) e -> p t e", p=128)
                if sbk == 0:
                    nc.sync.dma_start(xv[:, 0:1, :], src[:, 0:1, :])
                    nc.sync.dma_start(xv[:, 1:4, :], src[:, 1:4, :])
                else:
                    nc.sync.dma_start(xv, src)
                xu_tiles.append(xu)
                if sbk == 1:
                    load_w(0, ("wk", "wq"), wtmp0)
            load_w(0, ("wv", "wo"), wtmp0)

            pools = dict(sb=sb, sbt=sbt, ps=ps, dram=dram)
            _build_body(nc, tc, pools, xu_tiles, w_ext, wtmp0, load_w,
                        load_w8, ones_col, ident128, out_ext)

    nc.compile()
    return nc


def _build_body(nc, tc, pools, xu_tiles, w_ext, wtmp0, load_w, load_w8,
                ones_col, ident128, out_ext):
    sb, sbt, ps, dram = pools['sb'], pools['sbt'], pools['ps'], pools['dram']
    from contextlib import nullcontext

    SL = [slice(i * 512, (i + 1) * 512) for i in range(SB)]

    # ============ LayerNorm ============
    xn = {}
    for ec in range(EC):
        for sbk in range(SB):
            xn[ec, sbk] = sb.tile([128, 512], F32R, name=f"xn_{ec}_{sbk}")

    # the whole LN dependency spine runs in the high-priority queues so the
    # head-0 projections (normal queue) never sit behind a later s-block's
    # not-yet-ready transpose in engine FIFO order
    for sbk in range(SB):
        with tc.high_priority():
            xu = xu_tiles[sbk]
            st6 = sbt.tile([128, 4, 6], F32, name="st6", tag="st6", bufs=2)
            mv = sbt.tile([128, 4, 2], F32, name="mv", tag="mv", bufs=2)
            inv4 = sbt.tile([128, 4], F32, name="inv4", tag="inv4", bufs=2)
            va = sbt.tile([128, 4], F32, name="va", tag="va", bufs=2)
            vb = sbt.tile([128, 4], F32, name="vb", tag="vb", bufs=2)

            def seed_newton_xnu(jsl, jlist, eng=None):
                # rsqrt(v): v near 1, seed y0 = (1 + 1/v)/2 then one Newton
                # step (worst-case |v-1| ~ 0.5 -> < 7e-4 relative, below the
                # fp8-QK noise floor)
                eng = eng or nc.gpsimd
                nc.vector.tensor_scalar_add(vb[:, jsl], mv[:, jsl, 1], EPS)
                with nc.allow_low_precision("newton-polished below"):
                    nc.vector.reciprocal(inv4[:, jsl], vb[:, jsl])
                nc.vector.tensor_scalar(inv4[:, jsl], inv4[:, jsl], 0.5, 0.5,
                                        OP.mult, OP.add)
                for _ in range(1):
                    eng.tensor_mul(va[:, jsl], inv4[:, jsl], inv4[:, jsl])
                    eng.tensor_mul(va[:, jsl], va[:, jsl], vb[:, jsl])
                    eng.tensor_scalar(va[:, jsl], va[:, jsl], -0.5, 1.5,
                                      OP.mult, OP.add)
                    eng.tensor_mul(inv4[:, jsl], inv4[:, jsl], va[:, jsl])
                for j in jlist:
                    xnu = sbt.tile([128, E], F32, name="xnu", tag="xnu", bufs=3)
                    nc.vector.tensor_scalar(xnu[:], xu[:, j * E:(j + 1) * E],
                                            mv[:, j, 0:1], inv4[:, j:j + 1],
                                            OP.subtract, OP.mult)
                    for ec in range(EC):
                        # stps tag: unused until the first main loop, so the
                        # 32 LN transposes don't queue ahead of the head-0
                        # projections in the mm slot-grant FIFO
                        pt = ps.tile([128, 128], F32, name="pt", tag="stps",
                                     bufs=2)
                        nc.tensor.transpose(pt[:],
                                            xnu[:, ec * 128:(ec + 1) * 128],
                                            ident128[:])
                        nc.scalar.copy(xn[ec, sbk][:, j * 128:(j + 1) * 128],
                                       pt[:])

            if sbk == 0:
                # j0 fast path: its transpose gates the very first PE work
                nc.vector.bn_stats(st6[:, 0], xu[:, 0:E])
                nc.vector.bn_aggr(mv[:, 0], st6[:, 0])
                for j in range(1, 4):
                    nc.vector.bn_stats(st6[:, j], xu[:, j * E:(j + 1) * E])
                    nc.vector.bn_aggr(mv[:, j], st6[:, j])
                seed_newton_xnu(slice(0, 1), [0], eng=nc.vector)
                seed_newton_xnu(slice(1, 4), [1, 2, 3], eng=nc.vector)
            else:
                for j in range(4):
                    nc.vector.bn_stats(st6[:, j], xu[:, j * E:(j + 1) * E])
                    nc.vector.bn_aggr(mv[:, j], st6[:, j])
                seed_newton_xnu(slice(0, 4), [0, 1, 2, 3])

    def xn_col(ec, st):
        sbk, j = divmod(st, 4)
        return xn[ec, sbk][:, j * 128:(j + 1) * 128]

    # fp8 hi/lo split of xnT in DoubleRow (ec-paired) layout, produced on the
    # otherwise-idle Pool engine during head 0's f32r warmup; consumed by the
    # DR projections of heads 1..3
    xnph, xnpl = {}, {}
    if N_HEADS_BUILD > 1:
        for sbk in range(SB):
            xnph[sbk] = sb.tile([128, EC, 512], FP8, name=f"xnph_{sbk}")
            xnpl[sbk] = sb.tile([128, EC, 512], FP8, name=f"xnpl_{sbk}")
            for ec in range(EC):
                nc.gpsimd.tensor_copy(xnph[sbk][:, ec, :], xn[ec, sbk][:])
                nc.gpsimd.tensor_tensor(xnpl[sbk][:, ec, :], xn[ec, sbk][:],
                                        xnph[sbk][:, ec, :], OP.subtract)

    # ============ per-head attention ============
    acc = sb.tile([128, ST * E], F32, name="acc")

    bounce_in = [dram.tile([512, E], F32, name=f"bounce_in{i}",
                           tag=f"bin{i}", bufs=1) for i in range(SB)]
    bounce_view = [b.rearrange("(t p) e -> p t e", p=128) for b in bounce_in]

    st_h = {}

    def new_head_state(h, w):
        wr = {}
        w8 = {}
        for wname, wt in w.items():
            if wname.endswith("8"):
                w8[wname] = wt
                continue
            r = sbt.tile([128, EC * E], F32R, name=f"wr_{wname}", tag="wr",
                         bufs=6)
            if h == 0:
                nc.scalar.copy(r[:], wt[:])
            else:
                nc.vector.tensor_copy(r[:], wt[:])
            wr[wname] = r
        st_h[h] = dict(
            w=wr, w8=w8, kthi={}, ktlo={}, qthi={}, qtlo={}, vt={}, outT={},
            sq={},
            rowps=ps.tile([128, 64], F32, name="rowps", tag="rowps", bufs=1),
            ktbias=sbt.tile([128, 16], F32, name="ktbias", tag="ktbias", bufs=2),
            eq2=sbt.tile([128, 16], F32, name="eq2", tag="eq2", bufs=2),
        )

    def proj_mm(h, sbk):
        """K'/Q' projection blocks for one s-block.
        h==0: f32r tiles (kthi/qthi hold [128,512] f32r, no lo).
        h>0: fp8 hi+lo pair tiles [128, 2(ft), 512]."""
        s = st_h[h]
        for wname, hi_d, lo_d in (("wk", s['kthi'], s['ktlo']),
                                  ("wq", s['qthi'], s['qtlo'])):
            if h == 0:
                hi_t = {ft: sbt.tile([128, 512], F32R, name="kt0",
                                     tag="kthi" if wname == "wk" else "qthi",
                                     bufs=8)
                        for ft in range(EC)}
            else:
                pref = "kt" if wname == "wk" else "qt"
                hi_p = sbt.tile([128, 2, 512], FP8, name=f"{pref}hi",
                                tag="kthi" if wname == "wk" else "qthi", bufs=8)
                lo_p = sbt.tile([128, 2, 512], FP8, name=f"{pref}lo",
                                tag="ktlo" if wname == "wk" else "qtlo", bufs=8)
            for ft in range(EC):
                pp = ps.tile([128, 512], F32, name="pp", tag="mm", bufs=3)
                if h == 0:
                    wr = s['w'][wname]
                    for ec in range(EC):
                        o = ec * E + ft * 128
                        nc.tensor.matmul(pp[:], wr[:, o:o + 128],
                                         xn[ec, sbk][:],
                                         start=(ec == 0), stop=(ec == EC - 1))
                    nc.scalar.copy(hi_t[ft][:], pp[:])
                    src = hi_t[ft][:]
                else:
                    w8h = s['w8'][f"{wname}h8"][:].bitcast(FP8)
                    w8l = s['w8'][f"{wname}l8"][:].bitcast(FP8)
                    wsh = w8h[:, :, ft * 128:(ft + 1) * 128]
                    wsl = w8l[:, :, ft * 128:(ft + 1) * 128]
                    nc.tensor.matmul(pp[:], wsh, xnph[sbk][:],
                                     start=True, stop=False, perf_mode=DR)
                    nc.tensor.matmul(pp[:], wsh, xnpl[sbk][:],
                                     start=False, stop=False, perf_mode=DR)
                    nc.tensor.matmul(pp[:], wsl, xnph[sbk][:],
                                     start=False, stop=True, perf_mode=DR)
                    ktf = sbt.tile([128, 512], F32R, name="ktf", tag="ktf",
                                   bufs=4)
                    # 2^-8 undoes the host-side fp8 weight pre-scale
                    nc.vector.tensor_scalar_mul(ktf[:], pp[:], 1.0 / 256.0)
                    nc.vector.tensor_copy(hi_p[:, ft, :], ktf[:])
                    nc.vector.tensor_tensor(lo_p[:, ft, :], ktf[:],
                                            hi_p[:, ft, :], OP.subtract)
                    src = ktf[:]
                sqt = sbt.tile([128, 512], F32R, name="sqt", tag="sqc", bufs=6)
                if h == 0:
                    # warmup: Pool is busy with the LN newtons; DVE has slack
                    nc.vector.tensor_mul(sqt[:], src, src)
                else:
                    nc.gpsimd.tensor_mul(sqt[:], src, src)
                s['sq'][wname, ft, sbk] = sqt
            if h == 0:
                hi_d[sbk] = hi_t
            else:
                hi_d[sbk] = hi_p
                lo_d[sbk] = lo_p

    def proj_rs(h, sbk):
        """Row sums (k2/q2) for one s-block via stationary-squared matmuls,
        then the ACT bias column block and the exp(-q2/2) scale block."""
        s = st_h[h]
        rowps = s['rowps']
        ctx = nullcontext()
        ctx.__enter__()
        for qoff, wname in ((0, "wk"), (16, "wq")):
            sqs = [s['sq'].pop((wname, ft, sbk)) for ft in range(EC)]
            # complete each column's start/stop group back-to-back: a later
            # start=True re-marks the whole PSUM zero region, which would
            # drop a still-open group's first-pass accumulation
            for j in range(4):
                col = 2 * (qoff + sbk * 4 + j)
                for ft in range(EC):
                    nc.tensor.matmul(rowps[:, col:col + 2],
                                     sqs[ft][:, j * 128:(j + 1) * 128],
                                     ones_col[:],
                                     start=(ft == 0), stop=(ft == EC - 1))
        rv = rowps[:].rearrange("p (c two) -> p c two", two=2)
        c = slice(sbk * 4, (sbk + 1) * 4)
        nc.vector.tensor_scalar_mul(s['ktbias'][:, c], rv[:, c, 0], -0.5)
        nc.scalar.activation(s['eq2'][:, c],
                             rv[:, 16 + sbk * 4:16 + (sbk + 1) * 4, 0],
                             AF.Exp, scale=-0.5)
        ctx.__exit__(None, None, None)

    def emit_v(h, sbk):
        s = st_h[h]
        for st in range(sbk * 4, sbk * 4 + 4):
            pv = ps.tile([128, E], F32, name="pv", tag="mm", bufs=3)
            if h == 0:
                wv = s['w']['wv'][:]
                for ec in range(EC):
                    nc.tensor.matmul(pv[:], xn_col(ec, st),
                                     wv[:, ec * E:(ec + 1) * E],
                                     start=(ec == 0), stop=(ec == EC - 1))
            else:
                # 2^8 V scale is folded into this head's W_o on the host
                wvh = s['w8']["wvh8"][:].bitcast(FP8)
                wvl = s['w8']["wvl8"][:].bitcast(FP8)
                tb, j = divmod(st, 4)
                xsh = xnph[tb][:, :, j * 128:(j + 1) * 128]
                xsl = xnpl[tb][:, :, j * 128:(j + 1) * 128]
                nc.tensor.matmul(pv[:], xsh, wvh, start=True, stop=False,
                                 perf_mode=DR)
                nc.tensor.matmul(pv[:], xsh, wvl, start=False, stop=False,
                                 perf_mode=DR)
                nc.tensor.matmul(pv[:], xsl, wvh, start=False, stop=True,
                                 perf_mode=DR)
            v = sbt.tile([128, E], F32R, name="vt", tag="vt", bufs=26)
            if h == 0 and st % 2 == 1:
                nc.vector.tensor_copy(v[:], pv[:])
            else:
                nc.scalar.copy(v[:], pv[:])
            s['vt'][st] = v

    def emit_main(h, sbk):
        s = st_h[h]
        vt, ktbias = s['vt'], s['ktbias']

        ops = [ps.tile([128, 512], F32, name="ovps", tag=f"ovps{ft}", bufs=1)
               for ft in range(EC)]
        sc_q = {}
        SKEW = 4
        for tt in range(ST + SKEW):
            if tt < ST:
                tb, tj = divmod(tt, 4)
                stps = ps.tile([128, 512], F32, name="stps", tag="stps", bufs=2)
                if h == 0:
                    for ft in range(EC):
                        nc.tensor.matmul(stps[:],
                                         s['kthi'][tb][ft][:, tj * 128:(tj + 1) * 128],
                                         s['qthi'][sbk][ft][:],
                                         start=(ft == 0), stop=(ft == EC - 1))
                else:
                    khi = s['kthi'][tb][:, :, tj * 128:(tj + 1) * 128]
                    klo = s['ktlo'][tb][:, :, tj * 128:(tj + 1) * 128]
                    qhi = s['qthi'][sbk][:]
                    qlo = s['qtlo'][sbk][:]
                    nc.tensor.matmul(stps[:], khi, qhi, start=True, stop=False,
                                     perf_mode=DR)
                    nc.tensor.matmul(stps[:], khi, qlo, start=False, stop=False,
                                     perf_mode=DR)
                    nc.tensor.matmul(stps[:], klo, qhi, start=False, stop=True,
                                     perf_mode=DR)
                sc = sbt.tile([128, 512], F32R, name="sc", tag="sc", bufs=6)
                nc.scalar.activation(sc[:], stps[:], AF.Exp,
                                     bias=ktbias[:, tt:tt + 1], scale=1.0)
                sc_q[tt] = sc
            if tt >= SKEW:
                pv_tt = tt - SKEW
                sc_prev = sc_q.pop(pv_tt)
                for ft in range(EC):
                    nc.tensor.matmul(ops[ft][:],
                                     vt[pv_tt][:, ft * 128:(ft + 1) * 128],
                                     sc_prev[:],
                                     start=(pv_tt == 0), stop=(pv_tt == ST - 1))
        for ft in range(EC):
            o = sbt.tile([128, 512], F32R, name="outT", tag="outT", bufs=6)
            nc.vector.tensor_copy(o[:], ops[ft][:])
            s['outT'][ft, sbk] = o

    def emit_wo(h, sbk):
        s = st_h[h]
        wo = s['w']['wo'][:]
        for st in range(sbk * 4, sbk * 4 + 4):
            j = st % 4
            wops = ps.tile([128, E], F32, name="wops", tag="mm", bufs=3)
            for ft in range(EC):
                nc.tensor.matmul(wops[:],
                                 s['outT'][ft, sbk][:, j * 128:(j + 1) * 128],
                                 wo[:, ft * E:(ft + 1) * E],
                                 start=(ft == 0), stop=(ft == EC - 1))
            asl = acc[:, st * E:(st + 1) * E]
            eqcol = s['eq2'][:, st:st + 1]
            if h == 0:
                nc.vector.tensor_scalar(asl, wops[:], eqcol, None, OP.mult)
            else:
                nc.vector.scalar_tensor_tensor(asl, wops[:], eqcol,
                                               asl, OP.mult, OP.add)
        if h == N_HEADS_BUILD - 1:
            # two half-quarter DMAs: the first fires after its two acc
            # columns are scaled, shortening the post-compute tail
            for half in range(2):
                t0 = sbk * 4 + 2 * half
                nc.sync.dma_start(
                    bounce_view[sbk][:, 2 * half:2 * half + 2, :],
                    acc[:, t0 * E:(t0 + 2) * E]
                    .rearrange("p (t e) -> p t e", e=E))

    # ---- head 0: warmup emission interleaved with the LN tail ----
    if N_HEADS_BUILD > 0:
        new_head_state(0, wtmp0)
        proj_mm(0, 0)
        proj_mm(0, 1)
        proj_mm(0, 2)
        proj_mm(0, 3)
        emit_v(0, 0)
        proj_rs(0, 0)
        emit_v(0, 1)
        proj_rs(0, 1)
        emit_v(0, 2)
        proj_rs(0, 2)
        emit_v(0, 3)
        proj_rs(0, 3)

    for h in range(N_HEADS_BUILD):
        nxt = h + 1
        if nxt < N_HEADS_BUILD:
            wnxt = {}
            load_w(nxt, ("wo",), wnxt)
            load_w8(nxt, wnxt)
            new_head_state(nxt, wnxt)
        for sbk in range(SB):
            emit_main(h, sbk)
            emit_wo(h, sbk)
            if nxt < N_HEADS_BUILD:
                if sbk == 0:
                    proj_mm(nxt, 0)
                elif sbk == 1:
                    proj_mm(nxt, 1)
                    proj_rs(nxt, 0)
                    emit_v(nxt, 0)
                elif sbk == 2:
                    proj_mm(nxt, 2)
                    proj_rs(nxt, 1)
                    emit_v(nxt, 1)
                    proj_mm(nxt, 3)
                else:
                    proj_rs(nxt, 2)
                    emit_v(nxt, 2)
                    emit_v(nxt, 3)
                    proj_rs(nxt, 3)
        if h > 0:
            st_h.pop(h - 1, None)

    # ============ AllReduce over batch pair + store (4 quarters) ============
    for q in range(SB):
        osl = out_ext[q * 512:(q + 1) * 512, :]
        if NO_COLL:
            for half in range(2):
                nc.sync.dma_start(
                    out_ext[q * 512 + half * 256:(q * 512 + (half + 1) * 256), :],
                    bounce_in[q][half * 256:(half + 1) * 256, :])
        else:
            bo = dram.tile([512, E], F32, name=f"bounce_out{q}",
                           tag=f"bout{q}", bufs=1)
            nc.gpsimd.collective_compute(
                "AllReduce", OP.add,
                replica_groups=[[0, 1], [2, 3], [4, 5], [6, 7]],
                ins=[bounce_in[q].opt()],
                outs=[bo.opt()],
            )
            nc.sync.dma_start(osl, bo[:, :])


# ================= host side =================

W_SCALE = 256.0


def prep_inputs(x, ln_scale, W_q, W_k, W_v, W_o, gamma):
    """Build per-core input maps."""
    import ml_dtypes
    E4 = ml_dtypes.float8_e4m3
    x = np.asarray(x, np.float32)
    ln_scale = np.asarray(ln_scale, np.float32)
    W_q = np.asarray(W_q, np.float32)
    W_k = np.asarray(W_k, np.float32)
    W_v = np.asarray(W_v, np.float32)
    W_o = np.asarray(W_o, np.float32)
    gamma = np.asarray(gamma, np.float32).reshape(H)

    in_maps = []
    for c in range(N_CORES):
        b = c // 2
        h0 = HL * (c % 2)
        hs = list(range(h0, h0 + HL))
        g = gamma[hs]
        s2g = np.sqrt(2.0 * g).astype(np.float32)
        wq = (W_q[hs] * ln_scale[None, :, None] * s2g[:, None, None])
        wk = (W_k[hs] * ln_scale[None, :, None] * s2g[:, None, None])
        wv = (W_v[hs] * ln_scale[None, :, None])
        def _lay(w):   # [HL, E_in(=EC*128), E] -> [HL, 128, EC*E]
            return np.ascontiguousarray(
                w.reshape(HL, EC, 128, E).transpose(0, 2, 1, 3).reshape(HL, 128, EC * E))
        def _lay4(w):  # [HL, E_in, E] -> [HL, 128, EC, E]  (DoubleRow pairs)
            return np.ascontiguousarray(
                w.reshape(HL, EC, 128, E).transpose(0, 2, 1, 3))
        def _hilo(w):  # pre-scaled fp8 hi/lo bytes
            ws = (_lay4(w) * W_SCALE).astype(np.float32)
            hi = ws.astype(E4)
            lo = (ws - hi.astype(np.float32)).astype(E4)
            return hi.view(np.uint8), lo.view(np.uint8)
        wqh, wql = _hilo(wq)
        wkh, wkl = _hilo(wk)
        wvh, wvl = _hilo(wv)
        wo = _lay(np.stack([W_o[:, 256 * h:256 * (h + 1)].T.copy() for h in hs]))
        # heads 1..3 consume fp8 V scaled by 2^8; undo via their W_o slice
        wo = wo.copy()
        wo[1:] /= W_SCALE
        in_maps.append({
            "x": np.ascontiguousarray(x[b]),
            "wq": np.ascontiguousarray(_lay(wq)),
            "wk": np.ascontiguousarray(_lay(wk)),
            "wv": np.ascontiguousarray(_lay(wv)),
            "wo": np.ascontiguousarray(wo),
            "wqh8": wqh, "wql8": wql,
            "wkh8": wkh, "wkl8": wkl,
            "wvh8": wvh, "wvl8": wvl,
        })
    return in_maps


def assemble_output(results):
    out = np.empty((B, S, E), np.float32)
    for b in range(B):
        out[b] = results[2 * b]["out"]
    return out


_NC_CACHE = {}


def _get_nc():
    if 'nc' not in _NC_CACHE:
        _NC_CACHE['nc'] = build_kernel(debug=False)
    return _NC_CACHE['nc']


def kernel(x, e=None, p=None, ln_scale=None, W_q=None, W_k=None, W_v=None,
           W_o=None, gamma=None, **_unused):
    """Full-input entry point. e and p are unused by the reference network
    (use_ppe=False config); they are accepted and ignored."""
    in_maps = prep_inputs(x, ln_scale, W_q, W_k, W_v, W_o, gamma)
    nc = _get_nc()
    res = run_bass_kernel_spmd(nc, in_maps, core_ids=list(range(N_CORES)))
    return assemble_output(res.results)


# revision 57
# speedup vs baseline: 1.0158x; 1.0158x over previous
"""RBF-kernel attention (nn_Attention_76081050682051) on 8 TRN2 NeuronCores.

Self-contained Bass/Tile kernel. `kernel(**inputs)` takes the FULL unsharded
inputs of reference.setup_inputs() and returns the FULL [4, 2048, 256] f32
output.

Sharding (B x tensor-parallel heads): core c -> batch b = c//2, heads
[4*(c%2), 4*(c%2)+4); pairwise AllReduce ([0,1],[2,3],[4,5],[6,7]) combines
the two half-head partial outputs of each batch after the W_o projection.

Device math:
  LayerNorm per-partition via bn_stats/bn_aggr; rsqrt via DVE reciprocal +
  2 Newton steps (Pool engine); xnT blocks via PE transposes (f32r).
  Per head: K'T/Q'T = (folded W).T @ xnT with sqrt(2*gamma)*ln_scale folded
  into W_q/W_k on the host.  For heads 1-3 the K'/Q' blocks are split into
  fp8e4 hi+lo pairs and the S x S logits run as 3 DoubleRow fp8 matmuls
  (hi*hi + hi*lo + lo*hi, 256-deep contraction per pass) -- 0.75x the f32r
  cycle count at ~6e-3 relative error.  Head 0 stays f32r so its warmup
  overlaps the LayerNorm chain without extra DVE/ACT queue pressure.
  k2/q2 row sums: DVE/Pool squares the projection PSUM, then a [128,1]
  matmul with the squared tile as the *stationary* operand reduces 128
  t-values per instruction; results land per-partition, exactly the layout
  the ACT bias (k2) and the post-W_o scale (q2) need -- no DRAM round trip.
  scoresT[t, s] = exp(qk'[t,s] - k2'[t]/2) via one ACT op per [128,512]
  tile; exp(-q2'[s]/2) is applied after W_o as a per-partition scale.
  outT = V.T @ scoresT accumulates over t in PSUM; W_o runs on outT column
  slices; partial outputs AllReduce (4 quarter-chunks) within each batch
  pair.  Emission is software-pipelined across heads.
"""
import sys
sys.path.insert(0, '/opt/trn_rl_repo')
import numpy as np
from concourse import bass, bacc, tile, mybir, masks
from concourse.bass_utils import run_bass_kernel_spmd

F32 = mybir.dt.float32
F32R = mybir.dt.float32r
FP8 = mybir.dt.float8e4
AF = mybir.ActivationFunctionType
OP = mybir.AluOpType
DR = mybir.MatmulPerfMode.DoubleRow

B, S, E, H = 4, 2048, 256, 8
HL = 4          # heads per core
EC = 2          # e chunks of 128
SB = 4          # s blocks of 512
ST = 16         # s/t tiles of 128
N_CORES = 8
EPS = 1e-5

NO_COLL = False
N_HEADS_BUILD = HL


def build_kernel(debug=False):
    nc = bacc.Bacc("TRN2", target_bir_lowering=False, debug=False,
                   num_devices=N_CORES)

    x_ext = nc.declare_dram_parameter("x", [S, E], F32, isOutput=False)
    w_ext = {}
    for wname in ("wq", "wk", "wv", "wo"):
        w_ext[wname] = nc.declare_dram_parameter(wname, [HL, 128, EC * E], F32,
                                                 isOutput=False)
    # fp8 hi/lo weight pairs (heads 1..3), pre-scaled by 2^8 on the host so
    # the folded weights clear e4m3's subnormal band; stored as uint8 bytes
    for wname in ("wq", "wk", "wv"):
        for part in ("h", "l"):
            n = f"{wname}{part}8"
            w_ext[n] = nc.declare_dram_parameter(n, [HL, 128, EC, E],
                                                 mybir.dt.uint8, isOutput=False)
    out_ext = nc.declare_dram_parameter("out", [S, E], F32, isOutput=True)

    with tile.TileContext(nc) as tc:
        with tc.tile_pool(name="sb", bufs=1) as sb, \
             tc.tile_pool(name="sbt", bufs=1) as sbt, \
             tc.tile_pool(name="ps", bufs=1, space="PSUM") as ps, \
             tc.tile_pool(name="dram", bufs=1, space="DRAM") as dram:

            # ---------- constants ----------
            ones32 = sb.tile([128, 2], F32, name="ones32")
            nc.any.memset(ones32[:], 1.0)
            ones_col = sb.tile([128, 2], F32R, name="ones_col")
            nc.vector.tensor_copy(ones_col[:], ones32[:])
            ident128 = sb.tile([128, 128], F32, name="ident128")
            masks.make_identity(nc, ident128[:])
            # pre-warm the PE pstate ramp clock with cheap transposes so the
            # first real transposes/matmuls run at full frequency
            warm = ps.tile([128, 128], F32, name="warm", tag="stps", bufs=2)
            for _ in range(12):
                nc.tensor.transpose(warm[:32, :32], ident128[:32, :32],
                                    ident128[:32, :32])

            # ---------- x + head-0 weight loads (queue order matters) ----------
            xu_tiles = []
            wtmp0 = {}

            def load_w(h, names, store):
                for wname in names:
                    wt = sbt.tile([128, EC * E], F32, name=f"w_{wname}",
                                  tag="wtmp", bufs=4)
                    nc.sync.dma_start(wt[:], w_ext[wname][h])
                    store[wname] = wt

            def load_w8(h, store):
                for wname in ("wq", "wk", "wv"):
                    for part in ("h", "l"):
                        n = f"{wname}{part}8"
                        wt = sbt.tile([128, EC, E], mybir.dt.uint8,
                                      name=f"w_{n}", tag="w8", bufs=12)
                        nc.sync.dma_start(wt[:], w_ext[n][h])
                        store[n] = wt

            for sbk in range(SB):
                xu = sbt.tile([128, 4 * E], F32, name="xu", tag="xu", bufs=4)
                xv = xu[:].rearrange("p (t e) -> p t e", t=4)
                src = x_ext[sbk * 512:(sbk + 1) * 512, :] \
                    .rearrange("(t p) e -> p t e", p=128)
                if sbk == 0:
                    nc.sync.dma_start(xv[:, 0:1, :], src[:, 0:1, :])
                    nc.sync.dma_start(xv[:, 1:4, :], src[:, 1:4, :])
                else:
                    nc.sync.dma_start(xv, src)
                xu_tiles.append(xu)
                if sbk == 1:
                    load_w(0, ("wk", "wq"), wtmp0)
            load_w(0, ("wv", "wo"), wtmp0)

            pools = dict(sb=sb, sbt=sbt, ps=ps, dram=dram)
            _build_body(nc, tc, pools, xu_tiles, w_ext, wtmp0, load_w,
                        load_w8, ones_col, ident128, out_ext)

    nc.compile()
    return nc


def _build_body(nc, tc, pools, xu_tiles, w_ext, wtmp0, load_w, load_w8,
                ones_col, ident128, out_ext):
    sb, sbt, ps, dram = pools['sb'], pools['sbt'], pools['ps'], pools['dram']
    from contextlib import nullcontext

    SL = [slice(i * 512, (i + 1) * 512) for i in range(SB)]

    # ============ LayerNorm ============
    xn = {}
    for ec in range(EC):
        for sbk in range(SB):
            xn[ec, sbk] = sb.tile([128, 512], F32R, name=f"xn_{ec}_{sbk}")

    # the whole LN dependency spine runs in the high-priority queues so the
    # head-0 projections (normal queue) never sit behind a later s-block's
    # not-yet-ready transpose in engine FIFO order
    for sbk in range(SB):
        with tc.high_priority():
            xu = xu_tiles[sbk]
            st6 = sbt.tile([128, 4, 6], F32, name="st6", tag="st6", bufs=2)
            mv = sbt.tile([128, 4, 2], F32, name="mv", tag="mv", bufs=2)
            inv4 = sbt.tile([128, 4], F32, name="inv4", tag="inv4", bufs=2)
            va = sbt.tile([128, 4], F32, name="va", tag="va", bufs=2)
            vb = sbt.tile([128, 4], F32, name="vb", tag="vb", bufs=2)

            def seed_newton_xnu(jsl, jlist, eng=None):
                # rsqrt(v): v near 1, seed y0 = (1 + 1/v)/2 then one Newton
                # step (worst-case |v-1| ~ 0.5 -> < 7e-4 relative, below the
                # fp8-QK noise floor)
                eng = eng or nc.gpsimd
                nc.vector.tensor_scalar_add(vb[:, jsl], mv[:, jsl, 1], EPS)
                with nc.allow_low_precision("newton-polished below"):
                    nc.vector.reciprocal(inv4[:, jsl], vb[:, jsl])
                nc.vector.tensor_scalar(inv4[:, jsl], inv4[:, jsl], 0.5, 0.5,
                                        OP.mult, OP.add)
                for _ in range(1):
                    eng.tensor_mul(va[:, jsl], inv4[:, jsl], inv4[:, jsl])
                    eng.tensor_mul(va[:, jsl], va[:, jsl], vb[:, jsl])
                    eng.tensor_scalar(va[:, jsl], va[:, jsl], -0.5, 1.5,
                                      OP.mult, OP.add)
                    eng.tensor_mul(inv4[:, jsl], inv4[:, jsl], va[:, jsl])
                for j in jlist:
                    xnu = sbt.tile([128, E], F32, name="xnu", tag="xnu", bufs=3)
                    nc.vector.tensor_scalar(xnu[:], xu[:, j * E:(j + 1) * E],
                                            mv[:, j, 0:1], inv4[:, j:j + 1],
                                            OP.subtract, OP.mult)
                    for ec in range(EC):
                        # stps tag: unused until the first main loop, so the
                        # 32 LN transposes don't queue ahead of the head-0
                        # projections in the mm slot-grant FIFO
                        pt = ps.tile([128, 128], F32, name="pt", tag="stps",
                                     bufs=2)
                        nc.tensor.transpose(pt[:],
                                            xnu[:, ec * 128:(ec + 1) * 128],
                                            ident128[:])
                        nc.scalar.copy(xn[ec, sbk][:, j * 128:(j + 1) * 128],
                                       pt[:])

            if sbk == 0:
                # j0 fast path: its transpose gates the very first PE work
                nc.vector.bn_stats(st6[:, 0], xu[:, 0:E])
                nc.vector.bn_aggr(mv[:, 0], st6[:, 0])
                for j in range(1, 4):
                    nc.vector.bn_stats(st6[:, j], xu[:, j * E:(j + 1) * E])
                    nc.vector.bn_aggr(mv[:, j], st6[:, j])
                seed_newton_xnu(slice(0, 1), [0], eng=nc.vector)
                seed_newton_xnu(slice(1, 4), [1, 2, 3], eng=nc.vector)
            else:
                for j in range(4):
                    nc.vector.bn_stats(st6[:, j], xu[:, j * E:(j + 1) * E])
                    nc.vector.bn_aggr(mv[:, j], st6[:, j])
                seed_newton_xnu(slice(0, 4), [0, 1, 2, 3])

    def xn_col(ec, st):
        sbk, j = divmod(st, 4)
        return xn[ec, sbk][:, j * 128:(j + 1) * 128]

    # fp8 hi/lo split of xnT in DoubleRow (ec-paired) layout, produced on the
    # otherwise-idle Pool engine during head 0's f32r warmup; consumed by the
    # DR projections of heads 1..3
    xnph, xnpl = {}, {}
    if N_HEADS_BUILD > 1:
        for sbk in range(SB):
            xnph[sbk] = sb.tile([128, EC, 512], FP8, name=f"xnph_{sbk}")
            xnpl[sbk] = sb.tile([128, EC, 512], FP8, name=f"xnpl_{sbk}")
            for ec in range(EC):
                nc.gpsimd.tensor_copy(xnph[sbk][:, ec, :], xn[ec, sbk][:])
                nc.gpsimd.tensor_tensor(xnpl[sbk][:, ec, :], xn[ec, sbk][:],
                                        xnph[sbk][:, ec, :], OP.subtract)

    # ============ per-head attention ============
    acc = sb.tile([128, ST * E], F32, name="acc")

    bounce_in = [dram.tile([512, E], F32, name=f"bounce_in{i}",
                           tag=f"bin{i}", bufs=1) for i in range(SB)]
    bounce_view = [b.rearrange("(t p) e -> p t e", p=128) for b in bounce_in]

    st_h = {}

    def new_head_state(h, w):
        wr = {}
        w8 = {}
        for wname, wt in w.items():
            if wname.endswith("8"):
                w8[wname] = wt
                continue
            r = sbt.tile([128, EC * E], F32R, name=f"wr_{wname}", tag="wr",
                         bufs=6)
            if h == 0:
                nc.scalar.copy(r[:], wt[:])
            else:
                nc.vector.tensor_copy(r[:], wt[:])
            wr[wname] = r
        st_h[h] = dict(
            w=wr, w8=w8, kthi={}, ktlo={}, qthi={}, qtlo={}, vt={}, outT={},
            sq={},
            rowps=ps.tile([128, 64], F32, name="rowps", tag="rowps", bufs=1),
            ktbias=sbt.tile([128, 16], F32, name="ktbias", tag="ktbias", bufs=2),
            eq2=sbt.tile([128, 16], F32, name="eq2", tag="eq2", bufs=2),
        )

    def proj_mm(h, sbk):
        """K'/Q' projection blocks for one s-block.
        h==0: f32r tiles (kthi/qthi hold [128,512] f32r, no lo).
        h>0: fp8 hi+lo pair tiles [128, 2(ft), 512]."""
        s = st_h[h]
        for wname, hi_d, lo_d in (("wk", s['kthi'], s['ktlo']),
                                  ("wq", s['qthi'], s['qtlo'])):
            if h == 0:
                hi_t = {ft: sbt.tile([128, 512], F32R, name="kt0",
                                     tag="kthi" if wname == "wk" else "qthi",
                                     bufs=8)
                        for ft in range(EC)}
            else:
                pref = "kt" if wname == "wk" else "qt"
                hi_p = sbt.tile([128, 2, 512], FP8, name=f"{pref}hi",
                                tag="kthi" if wname == "wk" else "qthi", bufs=8)
                lo_p = sbt.tile([128, 2, 512], FP8, name=f"{pref}lo",
                                tag="ktlo" if wname == "wk" else "qtlo", bufs=8)
            for ft in range(EC):
                pp = ps.tile([128, 512], F32, name="pp", tag="mm", bufs=3)
                if h == 0:
                    wr = s['w'][wname]
                    for ec in range(EC):
                        o = ec * E + ft * 128
                        nc.tensor.matmul(pp[:], wr[:, o:o + 128],
                                         xn[ec, sbk][:],
                                         start=(ec == 0), stop=(ec == EC - 1))
                    nc.scalar.copy(hi_t[ft][:], pp[:])
                    src = hi_t[ft][:]
                else:
                    w8h = s['w8'][f"{wname}h8"][:].bitcast(FP8)
                    w8l = s['w8'][f"{wname}l8"][:].bitcast(FP8)
                    wsh = w8h[:, :, ft * 128:(ft + 1) * 128]
                    wsl = w8l[:, :, ft * 128:(ft + 1) * 128]
                    nc.tensor.matmul(pp[:], wsh, xnph[sbk][:],
                                     start=True, stop=False, perf_mode=DR)
                    nc.tensor.matmul(pp[:], wsh, xnpl[sbk][:],
                                     start=False, stop=False, perf_mode=DR)
                    nc.tensor.matmul(pp[:], wsl, xnph[sbk][:],
                                     start=False, stop=True, perf_mode=DR)
                    ktf = sbt.tile([128, 512], F32R, name="ktf", tag="ktf",
                                   bufs=4)
                    # 2^-8 undoes the host-side fp8 weight pre-scale
                    nc.vector.tensor_scalar_mul(ktf[:], pp[:], 1.0 / 256.0)
                    nc.vector.tensor_copy(hi_p[:, ft, :], ktf[:])
                    nc.vector.tensor_tensor(lo_p[:, ft, :], ktf[:],
                                            hi_p[:, ft, :], OP.subtract)
                    src = ktf[:]
                sqt = sbt.tile([128, 512], F32R, name="sqt", tag="sqc", bufs=6)
                if h == 0 or sbk == 3:
                    # h0 warmup: Pool is busy with LN newtons + xnp splits;
                    # sbk3: the slow Pool square would sit on the rs(h,3)
                    # chain that gates the next head's first main loop
                    nc.vector.tensor_mul(sqt[:], src, src)
                else:
                    nc.gpsimd.tensor_mul(sqt[:], src, src)
                s['sq'][wname, ft, sbk] = sqt
            if h == 0:
                hi_d[sbk] = hi_t
            else:
                hi_d[sbk] = hi_p
                lo_d[sbk] = lo_p

    def proj_rs(h, sbk):
        """Row sums (k2/q2) for one s-block via stationary-squared matmuls,
        then the ACT bias column block and the exp(-q2/2) scale block."""
        s = st_h[h]
        rowps = s['rowps']
        ctx = nullcontext()
        ctx.__enter__()
        for qoff, wname in ((0, "wk"), (16, "wq")):
            sqs = [s['sq'].pop((wname, ft, sbk)) for ft in range(EC)]
            # complete each column's start/stop group back-to-back: a later
            # start=True re-marks the whole PSUM zero region, which would
            # drop a still-open group's first-pass accumulation
            for j in range(4):
                col = 2 * (qoff + sbk * 4 + j)
                for ft in range(EC):
                    nc.tensor.matmul(rowps[:, col:col + 2],
                                     sqs[ft][:, j * 128:(j + 1) * 128],
                                     ones_col[:],
                                     start=(ft == 0), stop=(ft == EC - 1))
        rv = rowps[:].rearrange("p (c two) -> p c two", two=2)
        c = slice(sbk * 4, (sbk + 1) * 4)
        nc.vector.tensor_scalar_mul(s['ktbias'][:, c], rv[:, c, 0], -0.5)
        nc.scalar.activation(s['eq2'][:, c],
                             rv[:, 16 + sbk * 4:16 + (sbk + 1) * 4, 0],
                             AF.Exp, scale=-0.5)
        ctx.__exit__(None, None, None)

    def emit_v(h, sbk):
        s = st_h[h]
        for st in range(sbk * 4, sbk * 4 + 4):
            pv = ps.tile([128, E], F32, name="pv", tag="mm", bufs=3)
            if h == 0:
                wv = s['w']['wv'][:]
                for ec in range(EC):
                    nc.tensor.matmul(pv[:], xn_col(ec, st),
                                     wv[:, ec * E:(ec + 1) * E],
                                     start=(ec == 0), stop=(ec == EC - 1))
            else:
                # 2^8 V scale is folded into this head's W_o on the host
                wvh = s['w8']["wvh8"][:].bitcast(FP8)
                wvl = s['w8']["wvl8"][:].bitcast(FP8)
                tb, j = divmod(st, 4)
                xsh = xnph[tb][:, :, j * 128:(j + 1) * 128]
                xsl = xnpl[tb][:, :, j * 128:(j + 1) * 128]
                nc.tensor.matmul(pv[:], xsh, wvh, start=True, stop=False,
                                 perf_mode=DR)
                nc.tensor.matmul(pv[:], xsh, wvl, start=False, stop=False,
                                 perf_mode=DR)
                nc.tensor.matmul(pv[:], xsl, wvh, start=False, stop=True,
                                 perf_mode=DR)
            v = sbt.tile([128, E], F32R, name="vt", tag="vt", bufs=26)
            if h == 0 and st % 2 == 1:
                nc.vector.tensor_copy(v[:], pv[:])
            else:
                nc.scalar.copy(v[:], pv[:])
            s['vt'][st] = v

    def emit_main(h, sbk):
        s = st_h[h]
        vt, ktbias = s['vt'], s['ktbias']

        ops = [ps.tile([128, 512], F32, name="ovps", tag=f"ovps{ft}", bufs=1)
               for ft in range(EC)]
        sc_q = {}
        SKEW = 4
        for tt in range(ST + SKEW):
            if tt < ST:
                tb, tj = divmod(tt, 4)
                stps = ps.tile([128, 512], F32, name="stps", tag="stps", bufs=2)
                if h == 0:
                    for ft in range(EC):
                        nc.tensor.matmul(stps[:],
                                         s['kthi'][tb][ft][:, tj * 128:(tj + 1) * 128],
                                         s['qthi'][sbk][ft][:],
                                         start=(ft == 0), stop=(ft == EC - 1))
                else:
                    khi = s['kthi'][tb][:, :, tj * 128:(tj + 1) * 128]
                    klo = s['ktlo'][tb][:, :, tj * 128:(tj + 1) * 128]
                    qhi = s['qthi'][sbk][:]
                    qlo = s['qtlo'][sbk][:]
                    nc.tensor.matmul(stps[:], khi, qhi, start=True, stop=False,
                                     perf_mode=DR)
                    nc.tensor.matmul(stps[:], khi, qlo, start=False, stop=False,
                                     perf_mode=DR)
                    nc.tensor.matmul(stps[:], klo, qhi, start=False, stop=True,
                                     perf_mode=DR)
                sc = sbt.tile([128, 512], F32R, name="sc", tag="sc", bufs=6)
                nc.scalar.activation(sc[:], stps[:], AF.Exp,
                                     bias=ktbias[:, tt:tt + 1], scale=1.0)
                sc_q[tt] = sc
            if tt >= SKEW:
                pv_tt = tt - SKEW
                sc_prev = sc_q.pop(pv_tt)
                for ft in range(EC):
                    nc.tensor.matmul(ops[ft][:],
                                     vt[pv_tt][:, ft * 128:(ft + 1) * 128],
                                     sc_prev[:],
                                     start=(pv_tt == 0), stop=(pv_tt == ST - 1))
        for ft in range(EC):
            o = sbt.tile([128, 512], F32R, name="outT", tag="outT", bufs=6)
            nc.vector.tensor_copy(o[:], ops[ft][:])
            s['outT'][ft, sbk] = o

    def emit_wo(h, sbk):
        s = st_h[h]
        wo = s['w']['wo'][:]
        for st in range(sbk * 4, sbk * 4 + 4):
            j = st % 4
            wops = ps.tile([128, E], F32, name="wops", tag="mm", bufs=3)
            for ft in range(EC):
                nc.tensor.matmul(wops[:],
                                 s['outT'][ft, sbk][:, j * 128:(j + 1) * 128],
                                 wo[:, ft * E:(ft + 1) * E],
                                 start=(ft == 0), stop=(ft == EC - 1))
            asl = acc[:, st * E:(st + 1) * E]
            eqcol = s['eq2'][:, st:st + 1]
            if h == 0:
                nc.vector.tensor_scalar(asl, wops[:], eqcol, None, OP.mult)
            else:
                nc.vector.scalar_tensor_tensor(asl, wops[:], eqcol,
                                               asl, OP.mult, OP.add)
        if h == N_HEADS_BUILD - 1:
            # two half-quarter DMAs: the first fires after its two acc
            # columns are scaled, shortening the post-compute tail
            for half in range(2):
                t0 = sbk * 4 + 2 * half
                nc.sync.dma_start(
                    bounce_view[sbk][:, 2 * half:2 * half + 2, :],
                    acc[:, t0 * E:(t0 + 2) * E]
                    .rearrange("p (t e) -> p t e", e=E))

    # ---- head 0: warmup emission interleaved with the LN tail ----
    if N_HEADS_BUILD > 0:
        new_head_state(0, wtmp0)
        proj_mm(0, 0)
        proj_mm(0, 1)
        proj_mm(0, 2)
        proj_mm(0, 3)
        emit_v(0, 0)
        proj_rs(0, 0)
        emit_v(0, 1)
        proj_rs(0, 1)
        emit_v(0, 2)
        proj_rs(0, 2)
        emit_v(0, 3)
        proj_rs(0, 3)

    for h in range(N_HEADS_BUILD):
        nxt = h + 1
        if nxt < N_HEADS_BUILD:
            wnxt = {}
            load_w(nxt, ("wo",), wnxt)
            load_w8(nxt, wnxt)
            new_head_state(nxt, wnxt)
        for sbk in range(SB):
            emit_main(h, sbk)
            emit_wo(h, sbk)
            if nxt < N_HEADS_BUILD:
                if sbk == 0:
                    proj_mm(nxt, 0)
                elif sbk == 1:
                    proj_mm(nxt, 1)
                    proj_rs(nxt, 0)
                    emit_v(nxt, 0)
                elif sbk == 2:
                    proj_mm(nxt, 2)
                    proj_rs(nxt, 1)
                    emit_v(nxt, 1)
                else:
                    proj_mm(nxt, 3)
                    emit_v(nxt, 2)
                    proj_rs(nxt, 2)
                    emit_v(nxt, 3)
                    proj_rs(nxt, 3)
        if h > 0:
            st_h.pop(h - 1, None)

    # ============ AllReduce over batch pair + store (4 quarters) ============
    for q in range(SB):
        osl = out_ext[q * 512:(q + 1) * 512, :]
        if NO_COLL:
            for half in range(2):
                nc.sync.dma_start(
                    out_ext[q * 512 + half * 256:(q * 512 + (half + 1) * 256), :],
                    bounce_in[q][half * 256:(half + 1) * 256, :])
        else:
            bo = dram.tile([512, E], F32, name=f"bounce_out{q}",
                           tag=f"bout{q}", bufs=1)
            nc.gpsimd.collective_compute(
                "AllReduce", OP.add,
                replica_groups=[[0, 1], [2, 3], [4, 5], [6, 7]],
                ins=[bounce_in[q].opt()],
                outs=[bo.opt()],
            )
            nc.sync.dma_start(osl, bo[:, :])


# ================= host side =================

W_SCALE = 256.0


def prep_inputs(x, ln_scale, W_q, W_k, W_v, W_o, gamma):
    """Build per-core input maps."""
    import ml_dtypes
    E4 = ml_dtypes.float8_e4m3
    x = np.asarray(x, np.float32)
    ln_scale = np.asarray(ln_scale, np.float32)
    W_q = np.asarray(W_q, np.float32)
    W_k = np.asarray(W_k, np.float32)
    W_v = np.asarray(W_v, np.float32)
    W_o = np.asarray(W_o, np.float32)
    gamma = np.asarray(gamma, np.float32).reshape(H)

    in_maps = []
    for c in range(N_CORES):
        b = c // 2
        h0 = HL * (c % 2)
        hs = list(range(h0, h0 + HL))
        g = gamma[hs]
        s2g = np.sqrt(2.0 * g).astype(np.float32)
        wq = (W_q[hs] * ln_scale[None, :, None] * s2g[:, None, None])
        wk = (W_k[hs] * ln_scale[None, :, None] * s2g[:, None, None])
        wv = (W_v[hs] * ln_scale[None, :, None])
        def _lay(w):   # [HL, E_in(=EC*128), E] -> [HL, 128, EC*E]
            return np.ascontiguousarray(
                w.reshape(HL, EC, 128, E).transpose(0, 2, 1, 3).reshape(HL, 128, EC * E))
        def _lay4(w):  # [HL, E_in, E] -> [HL, 128, EC, E]  (DoubleRow pairs)
            return np.ascontiguousarray(
                w.reshape(HL, EC, 128, E).transpose(0, 2, 1, 3))
        def _hilo(w):  # pre-scaled fp8 hi/lo bytes
            ws = (_lay4(w) * W_SCALE).astype(np.float32)
            hi = ws.astype(E4)
            lo = (ws - hi.astype(np.float32)).astype(E4)
            return hi.view(np.uint8), lo.view(np.uint8)
        wqh, wql = _hilo(wq)
        wkh, wkl = _hilo(wk)
        wvh, wvl = _hilo(wv)
        wo = _lay(np.stack([W_o[:, 256 * h:256 * (h + 1)].T.copy() for h in hs]))
        # heads 1..3 consume fp8 V scaled by 2^8; undo via their W_o slice
        wo = wo.copy()
        wo[1:] /= W_SCALE
        in_maps.append({
            "x": np.ascontiguousarray(x[b]),
            "wq": np.ascontiguousarray(_lay(wq)),
            "wk": np.ascontiguousarray(_lay(wk)),
            "wv": np.ascontiguousarray(_lay(wv)),
            "wo": np.ascontiguousarray(wo),
            "wqh8": wqh, "wql8": wql,
            "wkh8": wkh, "wkl8": wkl,
            "wvh8": wvh, "wvl8": wvl,
        })
    return in_maps


def assemble_output(results):
    out = np.empty((B, S, E), np.float32)
    for b in range(B):
        out[b] = results[2 * b]["out"]
    return out


_NC_CACHE = {}


def _get_nc():
    if 'nc' not in _NC_CACHE:
        _NC_CACHE['nc'] = build_kernel(debug=False)
    return _NC_CACHE['nc']


def kernel(x, e=None, p=None, ln_scale=None, W_q=None, W_k=None, W_v=None,
           W_o=None, gamma=None, **_unused):
    """Full-input entry point. e and p are unused by the reference network
    (use_ppe=False config); they are accepted and ignored."""
    in_maps = prep_inputs(x, ln_scale, W_q, W_k, W_v, W_o, gamma)
    nc = _get_nc()
    res = run_bass_kernel_spmd(nc, in_maps, core_ids=list(range(N_CORES)))
    return assemble_output(res.results)
